# revision 68
# baseline (speedup 1.0000x reference)
"""Disentangled self-attention (DeBERTa-style) Trainium2 kernel, 8 NeuronCores.

Math restructuring (same as the 172us baseline): project q/k through
Wpk/Wpq instead of projecting pos_emb, so the single fp8 read of pos_emb
is the dominant data movement:

    c2p[h,i,j] = sum_c qpk[h,i,c] * pos[i,j,c]
    p2c[h,i,j] = sum_c kpq[h,j,c] * pos[j,i,c]   (+ k.bpq colbias term)

Each core owns 48 query rows; it computes its own c2p rows and the p2c
COLUMNS for every other core from the same pos read, exchanged via
AllToAll in 3 chunks.

Scheduling redesign vs the 172us baseline (trace-driven):
  * the collectives (dummy warm-up a2a + 3 real chunks) are the ONLY
    instructions on the gpsimd queue, and the dummy triggers at ~1us.
    Previously the blocking collective_compute stalled 48 per-slab
    SWDGE stores queued behind it, stretching the loop to ~135us.
  * all a2a-consuming work (g2 loads, p2c transposes, softmax) comes
    AFTER the full loop in per-engine program order; previously
    g2_load(0) sat at t=41 in the scalar queue and stalled the last
    loop slabs on AllToAll #0.
  * score columns are stored in a host-side permuted order
    X_n = (n%8)*48 + n//8, so a2a chunk k completes a CONTIGUOUS block
    of 8*n_slabs columns.  exp / probs-transpose / probs@v then run
    incrementally per chunk (unnormalized accumulate, one 1/sum scale
    at the end), hiding the old 33us tensor-bound tail inside the a2a
    waits; only the last 64 columns' work is exposed.
  * c2p rows ride SBUF->SBUF HWDGE extracts straight out of the cast
    staging tile (no DRAM round trip); the colbias broadcast is a K=1
    ones-matmul accumulated into the c2c PSUM (no SWDGE broadcast).
"""

import sys

sys.path.insert(0, "/opt/trn_rl_repo")

import math
import numpy as np
import ml_dtypes

import concourse.bass as bass
import concourse.bacc as bacc
import concourse.mybir as mybir
import concourse.tile as tile
from concourse.bass_utils import run_bass_kernel_spmd

BF16 = mybir.dt.bfloat16
F8E3 = mybir.dt.float8e3
F32 = mybir.dt.float32
AF = mybir.ActivationFunctionType
ADD = mybir.AluOpType.add

S = 384
H = 768
NH = 12
D = 64
NC = 8
TB = S // NC  # 48 rows per core
NCH = H // 128  # 6 chunks of the hidden dim
GT = 4  # t-slabs per pos DMA group (= PE column-tile width)
NG = TB // GT  # 12 groups
CHUNKS = [(0, 22), (22, 22), (44, 4)]  # a2a chunks: (t_off, n_slabs)
# Wpk/Wpq are ~N(0, 0.02^2): prescaled by 2^6 on the host so fp8 e3m4
# stays in its normal range; compensated in the exp scale (both stg
# halves are qkp-derived).
WPSCALE = 64.0


def build_module():
    nc = bacc.Bacc(trn_type="TRN2", num_devices=NC, debug=False)

    # ---- I/O ----
    pos_d = nc.dram_tensor("pos", [NG, 128, GT, NCH, S], F8E3, kind="ExternalInput")
    hsTall_d = nc.dram_tensor("hsTall", [128, NCH, S + TB], BF16, kind="ExternalInput")
    wq_d = nc.dram_tensor("wq", [128, NCH, H], BF16, kind="ExternalInput")
    wk_d = nc.dram_tensor("wk", [128, NCH, H], BF16, kind="ExternalInput")
    wv_d = nc.dram_tensor("wv", [128, NCH, H], BF16, kind="ExternalInput")
    wpkT_d = nc.dram_tensor("wpkT", [128, NCH, H], F8E3, kind="ExternalInput")
    wpqT_d = nc.dram_tensor("wpqT", [128, NCH, H], F8E3, kind="ExternalInput")
    bqT_d = nc.dram_tensor("bqT", [128, NCH], F32, kind="ExternalInput")
    bkT_d = nc.dram_tensor("bkT", [128, NCH], F32, kind="ExternalInput")
    bv_d = nc.dram_tensor("bv", [H], BF16, kind="ExternalInput")
    bpqd_d = nc.dram_tensor("bpqd", [128, NCH, NH], BF16, kind="ExternalInput")
    mask_d = nc.dram_tensor("maskrow", [S], BF16, kind="ExternalInput")
    ident_d = nc.dram_tensor("ident", [128, 128], BF16, kind="ExternalInput")
    out_d = nc.dram_tensor("out", [TB, H], F32, kind="ExternalOutput")

    with tile.TileContext(nc) as tc:
        with (
            tc.tile_pool(name="const", bufs=1) as cpool,
            tc.tile_pool(name="work", bufs=1) as wpool,
            tc.tile_pool(name="posT", bufs=3) as ppool,
            tc.tile_pool(name="a2asb", bufs=1) as apool,
            tc.tile_pool(name="g2p", bufs=1) as gpool,
            tc.tile_pool(name="psum", bufs=3, space="PSUM") as pspool,
            tc.tile_pool(name="psum2", bufs=5, space="PSUM") as ps2pool,
            tc.tile_pool(name="dram", bufs=1, space="DRAM") as dpool,
        ):
            # ---- startup DMAs.  pos rides the Sync ring; weights ride the
            # Scalar HWDGE ring ordered by need-time.  The gpsimd queue
            # carries ONLY the broadcasts it alone can do, then the
            # collectives -- nothing may queue behind a blocking
            # collective_compute.
            ident = cpool.tile([128, 128], BF16, tag="ident")
            wq = cpool.tile([128, NCH, H], BF16, tag="wq")
            wk = cpool.tile([128, NCH, H], BF16, tag="wk")
            wpkT = cpool.tile([128, NCH, H], F8E3, tag="wpkT")
            wpqT = cpool.tile([128, NCH, H], F8E3, tag="wpqT")
            hsTall = cpool.tile([128, NCH, S + TB], BF16, tag="hsTall")
            wv = cpool.tile([128, NCH, H], BF16, tag="wv")
            bqT = cpool.tile([128, NCH], F32, tag="bqT")
            bkT = cpool.tile([128, NCH], F32, tag="bkT")
            bpqd = cpool.tile([128, NCH, NH], BF16, tag="bpqd")
            mask_p0 = cpool.tile([1, S], BF16, tag="mask_p0")
            nc.sync.dma_start(wq[:], wq_d[:])
            nc.sync.dma_start(hsTall[:], hsTall_d[:])
            nc.sync.dma_start(wk[:], wk_d[:])

            # ---- dummy-collective staging FIRST (scalar ring head), so
            # the gpsimd dummy AllToAll triggers at ~1.5us and pays the
            # ~40-50us barrier + first-collective cost overlapped with
            # the startup DMAs.  gpsimd carries ONLY collective_computes
            # -- nothing may queue behind a blocking collective, and the
            # ~10us SWDGE descriptor-gen of a broadcast ahead of the
            # trigger costs the whole chain that delay.
            qbd = wpool.tile([128, NCH, 2 * TB], BF16, tag="qbd")
            kbd = wpool.tile([128, NCH, 2 * TB], BF16, tag="kbd")
            ones48 = cpool.tile([1, TB], BF16, tag="ones48")
            onesNH = cpool.tile([1, NH], BF16, tag="onesNH")
            ones128 = cpool.tile([1, 128], BF16, tag="ones128")
            bv_p0 = cpool.tile([1, H], BF16, tag="bv_p0")
            # No warm-up dummy collective: chunk0's data is staged by
            # ~30us, well before the ~65-95us ncfw first-collective
            # floor, so the first real AllToAll absorbs the barrier +
            # launch skew itself.
            nc.vector.memset(qbd[:], 0.0)
            nc.vector.memset(kbd[:], 0.0)
            nc.vector.memset(ones48[:], 1.0)
            nc.vector.memset(onesNH[:], 1.0)
            nc.vector.memset(ones128[:], 1.0)

            # first three pos groups issue ahead of ident (transposes need
            # ident only post-loop); later groups issue from the loop.
            NPRE = 3
            posT_pre = []
            for g in range(NPRE):
                pt = ppool.tile([128, GT, NCH, S], F8E3, tag="posT", name="posT")
                nc.sync.dma_start(pt[:], pos_d[g])
                posT_pre.append(pt)
            nc.sync.dma_start(ident[:], ident_d[:])

            # scalar ring ordered by need-time: tiny biases, then the qkp
            # weights (the whole PE backlog -- and with it the pos-buffer
            # recycling -- waits on these), then the later-need hsT/wv.
            nc.scalar.dma_start(bqT[:], bqT_d[:])
            nc.scalar.dma_start(bkT[:], bkT_d[:])
            nc.scalar.dma_start(wpkT[:], wpkT_d[:])
            nc.scalar.dma_start(wpqT[:], wpqT_d[:])
            nc.scalar.dma_start(bpqd[:], bpqd_d[:])
            nc.scalar.dma_start(bv_p0[:], bv_d[:])
            nc.scalar.dma_start(mask_p0[:], mask_d[:])
            nc.scalar.dma_start(wv[:], wv_d[:])

            # ---- PE warm-up junk so HAM unthrottles during the DMA wait
            # (K=1 on the memset ones vector -- no dependency on loads)
            psw = ps2pool.tile([128, 128], F32, tag="ps2")
            for _ in range(50):
                nc.tensor.matmul(psw[:], ones128[:], ones128[:])

            # ---- own-row projections qT_own / kT_own ----
            qTo = wpool.tile([128, NCH, TB], BF16, tag="qTo")
            for m in range(NCH):
                pso = ps2pool.tile([128, TB], F32, tag="ps2")
                for c in range(NCH):
                    nc.tensor.matmul(
                        pso[:], wq[:, c, m * 128 : (m + 1) * 128], hsTall[:, c, S : S + TB],
                        start=(c == 0), stop=(c == NCH - 1),
                    )
                nc.vector.tensor_scalar_add(qTo[:, m, :], pso[:], bqT[:, m : m + 1])

            # ---- k projection: all X-ordered columns AND the own rows in
            # one matmul per (m, c) (hsTall carries both column sets), so
            # no separate kT filler units sit in the loop's PE FIFO.
            kTall = wpool.tile([128, NCH, S + TB], BF16, tag="kTall")
            for m in range(NCH):
                psk = ps2pool.tile([128, S + TB], F32, tag="ps2")
                for c in range(NCH):
                    nc.tensor.matmul(
                        psk[:], wk[:, c, m * 128 : (m + 1) * 128], hsTall[:, c, :],
                        start=(c == 0), stop=(c == NCH - 1),
                    )
                nc.vector.tensor_scalar_add(kTall[:, m, :], psk[:], bkT[:, m : m + 1])

            # ---- block-diagonal q/k for the per-head pos projections ----
            for mh in range(NCH):
                nc.vector.tensor_copy(qbd[0:64, mh, 0:96:2], qTo[0:64, mh, :])
                nc.vector.tensor_copy(qbd[64:128, mh, 1:96:2], qTo[64:128, mh, :])
                nc.vector.tensor_copy(kbd[0:64, mh, 0:96:2], kTall[0:64, mh, S : S + TB])
                nc.vector.tensor_copy(kbd[64:128, mh, 1:96:2], kTall[64:128, mh, S : S + TB])

            # ---- qkp[128, m, t, 24]: cols 0:12 kpq (p2c side), 12:24 qpk
            # (c2p side).  p2c occupies matmul-output rows 32j+0:12 so the
            # strided deinterleave copy reads at a legal engine base; the
            # c2p rows 32j+12:24 leave via DMA (base-12 DMA reads legal).
            qkp = wpool.tile([128, NCH, TB, 2 * NH], BF16, tag="qkp")
            for m in range(NCH):
                for mh in range(NCH):
                    ps1 = ps2pool.tile([128, 2 * TB], F32, tag="ps2")
                    nc.tensor.matmul(
                        ps1[:], wpkT[:, mh, m * 128 : (m + 1) * 128], qbd[:, mh, :]
                    )
                    src1 = ps1[:].rearrange("p (t two) -> p t two", two=2)
                    if mh % 2 == 0:
                        nc.scalar.activation(
                            qkp[:, m, :, NH + 2 * mh : NH + 2 * mh + 2], src1, AF.Copy
                        )
                    else:
                        nc.vector.tensor_copy(
                            qkp[:, m, :, NH + 2 * mh : NH + 2 * mh + 2], src1
                        )
                    ps2 = ps2pool.tile([128, 2 * TB], F32, tag="ps2")
                    nc.tensor.matmul(
                        ps2[:], wpqT[:, mh, m * 128 : (m + 1) * 128], kbd[:, mh, :]
                    )
                    src2 = ps2[:].rearrange("p (t two) -> p t two", two=2)
                    if mh % 2 == 0:
                        nc.vector.tensor_copy(
                            qkp[:, m, :, 2 * mh : 2 * mh + 2], src2
                        )
                    else:
                        nc.scalar.activation(
                            qkp[:, m, :, 2 * mh : 2 * mh + 2], src2, AF.Copy
                        )

            # ---- tiles for the main loop + tail ----
            # per-head v columns with a ones column appended: the probs@v
            # matmul's 65th output column is then the softmax partial sum
            # for free (no ACT accum_out / READ_ACCUMULATOR per head).
            v_sb = wpool.tile([128, 3, NH, D + 1], BF16, tag="v_sb")
            nc.vector.memset(v_sb[:, :, :, D : D + 1], 1.0)
            scores = wpool.tile([TB, NH, S], F32, tag="scores")
            colbias = wpool.tile([NH, S], BF16, tag="colbias")
            cbp0 = wpool.tile([1, NH, S], BF16, tag="cbp0")
            # c2p rows in plain stream (X) order -> contiguous score add.
            # One DRAM store per 4-slab group (a dma_start costs ~0.8us of
            # ENGINE time regardless of size, so 12 batched stores beat 48
            # per-slab SBUF->SBUF extracts), reloaded in 2 halves.
            c2p_rows = wpool.tile([TB, NH, S], BF16, tag="c2p_rows")
            c2p_dram = dpool.tile([TB, NH, S], BF16, name="c2p_dram")
            # p2c send staging [h, dest, t_local, i_local], filled by a
            # strided deinterleave copy from the per-group cast tile,
            # staged per chunk to a2a_in DRAM.
            comb = [None, None, None]
            a2a_in = [
                dpool.tile([NC, NH, n, TB], BF16, name=f"a2a_in{k}")
                for k, (off, n) in enumerate(CHUNKS)
            ]
            a2a_out = [
                dpool.tile([NC, NH, n, TB], BF16, name=f"a2a_out{k}")
                for k, (off, n) in enumerate(CHUNKS)
            ]
            g2 = [None, None, None]

            def alloc_comb(k):
                n = CHUNKS[k][1]
                tag = "a2aAB" if k < 2 else "a2aC"
                comb[k] = apool.tile(
                    [NH, NC, n, TB], BF16, tag=tag, name=f"comb{k}"
                )

            def v_unit(jc, nh):
                def run():
                    ps = ps2pool.tile([128, S], F32, tag="ps2")
                    for c in range(NCH):
                        nc.tensor.matmul(
                            ps[:],
                            hsTall[:, c, jc * 128 : (jc + 1) * 128],
                            wv[:, c, nh * S : (nh + 1) * S],
                            start=(c == 0), stop=False,
                        )
                    # bias broadcast across the 128 j-rows via a K=1
                    # ones-matmul accumulate (no SWDGE broadcast needed).
                    nc.tensor.matmul(
                        ps[:], ones128[:], bv_p0[:, nh * S : (nh + 1) * S],
                        start=False, stop=True,
                    )
                    nc.scalar.activation(
                        v_sb[:, jc, 6 * nh : 6 * (nh + 1), 0:D],
                        ps[:].rearrange("p (h d) -> p h d", h=6),
                        AF.Copy,
                    )
                return run

            def kb_unit():
                # colbias[h, j] = bpq_h . k_j  (+ mask*sqrt(D), via a K=1
                # ones-matmul).  Copied to partition 0 so c2c can add it
                # with another K=1 accumulate -- no SWDGE broadcast.
                pskb = ps2pool.tile([NH, S], F32, tag="ps2")
                nc.tensor.matmul(
                    pskb[:], onesNH[:], mask_p0[:], start=True, stop=False,
                )
                for m in range(NCH):
                    nc.tensor.matmul(
                        pskb[:], bpqd[:, m, :], kTall[:, m, 0:S],
                        start=False, stop=(m == NCH - 1),
                    )
                nc.vector.tensor_copy(colbias[:], pskb[:])
                nc.scalar.dma_start(cbp0[:], colbias[:])

            def c2c_unit(h):
                def run():
                    mh, oh = h // 2, (h % 2) * 64
                    ps = ps2pool.tile([TB, S], F32, tag="ps2")
                    nc.tensor.matmul(
                        ps[:], qTo[oh : oh + 64, mh, :], kTall[oh : oh + 64, mh, 0:S],
                        start=True, stop=False,
                    )
                    nc.tensor.matmul(
                        ps[:], ones48[:], cbp0[:, h, :], start=False, stop=True,
                    )
                    if h % 2 == 0:
                        nc.scalar.activation(scores[:, h, :], ps[:], AF.Copy)
                    else:
                        nc.vector.tensor_copy(scores[:, h, :], ps[:])
                return run

            def c2p_reload(half):
                def run():
                    lo, hi = (0, 32) if half == 0 else (32, TB)
                    nc.scalar.dma_start(c2p_rows[lo:hi], c2p_dram[lo:hi])
                return run

            def exp_unit(h):
                # probs = exp(c2c + colbias); the c2p / p2c factors are
                # exp'd at the per-slab cast and MULTIPLIED in afterwards
                # (exp(a+b+c) = exp(a)exp(b)exp(c)), so nothing in the
                # softmax waits on the AllToAll except the final product.
                def run():
                    nc.scalar.activation(
                        probs[:, h, :], scores[:, h, :], AF.Exp, scale=isqd
                    )
                return run

            def c2p_mul(half):
                def run():
                    lo, hi = (0, 32) if half == 0 else (32, TB)
                    nc.vector.tensor_tensor(
                        probs[lo:hi], probs[lo:hi], c2p_rows[lo:hi],
                        op=mybir.AluOpType.mult,
                    )
                return run

            def stage_a2a(k):
                nc.scalar.dma_start(
                    a2a_in[k][:].rearrange("d h t i -> h d t i"),
                    comb[k][:],
                )

            def cc_issue(k):
                nc.gpsimd.collective_compute(
                    "AllToAll",
                    mybir.AluOpType.bypass,
                    replica_groups=[list(range(NC))],
                    ins=[a2a_in[k].opt()],
                    outs=[a2a_out[k].opt()],
                )

            def g2_load(k):
                n = CHUNKS[k][1]
                tag = "g2AB" if k < 2 else "g2C"
                g2[k] = gpool.tile(
                    [NC * NH, n, TB], BF16, tag=tag, name=f"g2_{k}"
                )
                nc.scalar.dma_start(
                    g2[k][:],
                    a2a_out[k][:].rearrange("d h t i -> (d h) t i"),
                )

            sums = wpool.tile([TB, NH], F32, tag="sums")
            recip = wpool.tile([TB, NH], F32, tag="recip")
            probs = wpool.tile([TB, NH, S], BF16, tag="probs")
            ptile = wpool.tile([128, 3, NH, TB], BF16, tag="ptile")
            # out_acc[:, h, 0:64] accumulates probs@v; column 64 (from the
            # appended ones column of v) accumulates the softmax sums.
            out_acc = wpool.tile([TB, NH, D + 1], F32, tag="out_acc")
            out_sb = wpool.tile([TB, H], F32, tag="out_sb")
            isqd = 1.0 / math.sqrt(D)

            def p2c_pair(k, tl):
                # transpose 2 slabs into PSUM and multiply the exp'd p2c
                # factors straight into probs from there (DVE reads PSUM;
                # no staging tile, no drain copy).
                off, n = CHUNKS[k]
                pst2 = ps2pool.tile([TB, 2, NC * NH], BF16, tag="ps2")
                for q in range(2):
                    nc.tensor.transpose(
                        pst2[:, q, :], g2[k][:, tl + q, :],
                        ident[0 : NC * NH, 0 : NC * NH],
                    )
                c0 = NC * (off + tl)
                pr = probs[:, :, c0 : c0 + 2 * NC].rearrange(
                    "i h (t s) -> i h t s", s=NC
                )
                nc.vector.tensor_tensor(
                    pr,
                    pr,
                    pst2[:].rearrange("i t (d h) -> i h t d", d=NC),
                    op=mybir.AluOpType.mult,
                )

            # ---- filler schedule keyed by global t ----
            filler = {}
            filler.setdefault(1, []).append(kb_unit)
            for h in range(NH):
                filler.setdefault(2 + h, []).append(c2c_unit(h))  # t = 2..13
            for h in range(NH):
                filler.setdefault(4 + h, []).append(exp_unit(h))  # t = 4..15
            filler.setdefault(35, []).append(c2p_reload(0))
            filler.setdefault(41, []).append(c2p_mul(0))

            # ---- main loop over 4-slab groups ----
            for g in range(NG):
                if g < NPRE:
                    posT = posT_pre[g]
                else:
                    posT = ppool.tile([128, GT, NCH, S], F8E3, tag="posT", name="posT")
                    nc.sync.dma_start(posT[:], pos_d[g])
                ps = pspool.tile([128, S], F32, tag="ps")
                stg = ppool.tile([2 * NH, GT, S], BF16, tag="stg", name="stg")
                for j in range(GT):
                    t = GT * g + j
                    for k, (off, n) in enumerate(CHUNKS):
                        if t == off:
                            alloc_comb(k)
                for m in range(NCH):
                    for j in range(GT):
                        t = GT * g + j
                        nc.tensor.matmul(
                            ps[32 * j : 32 * j + 2 * NH, :],
                            qkp[:, m, t, :],
                            posT[:, j, m, :],
                            start=(m == 0),
                            stop=(m == NCH - 1),
                            tile_position=(0, 32 * j),
                            # the sim's zero-region tracker ignores the
                            # partition base, so the 4 disjoint column
                            # groups falsely collide; HW has_written is
                            # per-element.
                            skip_group_check=True,
                        )
                for j in range(GT):
                    t = GT * g + j
                    for k, (off, n) in enumerate(CHUNKS):
                        if off <= t < off + n:
                            break
                    tl = t - off
                    # exp'd cast off PSUM (the c2p/p2c softmax factors are
                    # exchanged and stored pre-exponentiated); the p2c half
                    # (rows 0:12) deinterleaves stream position n = i*8 + d
                    # into (dest d, dest-local row i) with a strided DVE
                    # copy, the c2p half (rows 12:24) leaves by DMA.
                    nc.scalar.activation(
                        stg[:, j, :], ps[32 * j : 32 * j + 2 * NH, :], AF.Exp,
                        scale=isqd / WPSCALE,
                    )
                    nc.vector.tensor_copy(
                        comb[k][:, :, tl, :],
                        stg[0:NH, j, :].rearrange("h (i d) -> h d i", d=NC),
                    )
                    if j == GT - 1:
                        # NOT on the sync queue: a data-dependent issue
                        # there would stall subsequent pos-prefetch issues.
                        nc.scalar.dma_start(
                            c2p_dram[GT * g : GT * (g + 1)].rearrange(
                                "t h s -> h t s"
                            ),
                            stg[NH : 2 * NH, :, :],
                        )
                    if t == CHUNKS[0][0] + CHUNKS[0][1] - 1:
                        stage_a2a(0)
                        cc_issue(0)
                    if t == CHUNKS[1][0] + CHUNKS[1][1] - 1:
                        stage_a2a(1)
                        cc_issue(1)
                    for f in filler.get(t, []):
                        f()

            # ---- after the loop: last a2a + c2p second half ----
            stage_a2a(2)
            cc_issue(2)
            c2p_reload(1)()
            c2p_mul(1)()
            for jc in range(3):
                for nh in range(2):
                    v_unit(jc, nh)()

            # ---- incremental probs@v per a2a chunk window.  probs
            # already holds exp(c2c+colbias)*exp(c2p); chunk k's exp'd
            # p2c factors multiply in, completing columns [160k, 160k+8n)
            # contiguous (X order) = 128-column block jc=k plus carry,
            # then transpose + accumulate probs@v into out_acc.
            for k, (off, n) in enumerate(CHUNKS):
                g2_load(k)
                # only the pairs covering jc-block k gate this window's
                # transposes; trailing pairs (feeding window k+1) overlap.
                need = max(0, min(n // 2, (128 * (k + 1) - NC * off + 15) // 16))
                for pr in range(need):
                    p2c_pair(k, 2 * pr)
                for h in range(NH):
                    pst = ps2pool.tile([128, TB], BF16, tag="ps2")
                    nc.tensor.transpose(
                        pst[:], probs[:, h, k * 128 : (k + 1) * 128],
                        ident[0:TB, 0:TB],
                    )
                    if h % 2 == 0:
                        nc.vector.tensor_copy(ptile[:, k, h, :], pst[:])
                    else:
                        nc.scalar.activation(ptile[:, k, h, :], pst[:], AF.Copy)
                    psc = ps2pool.tile([TB, D + 1], F32, tag="ps2")
                    nc.tensor.matmul(
                        psc[:], ptile[:, k, h, :], v_sb[:, k, h, :],
                    )
                    if k == 0:
                        if h % 2 == 0:
                            nc.scalar.activation(out_acc[:, h, :], psc[:], AF.Copy)
                        else:
                            nc.vector.tensor_copy(out_acc[:, h, :], psc[:])
                    else:
                        nc.vector.tensor_tensor(
                            out_acc[:, h, :], out_acc[:, h, :], psc[:], op=ADD
                        )
                    if k == 2:
                        # finalize per head as soon as its last PV lands
                        nc.vector.reciprocal(
                            recip[:, h : h + 1], out_acc[:, h, D : D + 1]
                        )
                        nc.scalar.activation(
                            out_sb[:, h * D : (h + 1) * D],
                            out_acc[:, h, 0:D], AF.Copy,
                            scale=recip[:, h : h + 1],
                        )
                for pr in range(need, n // 2):
                    p2c_pair(k, 2 * pr)

            nc.sync.dma_start(out_d[:], out_sb[:])

    nc.compile()
    return nc


_NC_CACHE = None


def _chunked(w):
    """[H, X] f32 -> [128, NCH, X] bf16 with [p, m, x] = w[128m+p, x]."""
    bf = ml_dtypes.bfloat16
    X = w.shape[1]
    return np.ascontiguousarray(
        np.asarray(w, np.float32).reshape(NCH, 128, X).transpose(1, 0, 2)
    ).astype(bf)


def _chunked_f8(w):
    f8 = ml_dtypes.float8_e3m4
    X = w.shape[1]
    return np.ascontiguousarray(
        np.asarray(w, np.float32).reshape(NCH, 128, X).transpose(1, 0, 2)
    ).astype(f8)


def _prep_inputs(hidden_states, attention_mask, pos_emb, Wq, bq, Wk, bk, Wv, bv,
                 Wpk, bpk, Wpq, bpq):
    bf = ml_dtypes.bfloat16
    f8 = ml_dtypes.float8_e3m4
    # column permutation: stream position n holds key index X_n so that
    # a2a chunk k completes a contiguous block of score columns.
    X = (np.arange(S) % NC) * TB + np.arange(S) // NC
    hs = np.ascontiguousarray(np.asarray(hidden_states, np.float32)[0])  # (S, H)
    hsT_nat = np.ascontiguousarray(hs.T)  # (H, S) f32, natural order
    hsT_X = np.ascontiguousarray(hsT_nat[:, X])
    bqT = np.ascontiguousarray(np.asarray(bq, np.float32).reshape(NCH, 128).T)
    bkT = np.ascontiguousarray(np.asarray(bk, np.float32).reshape(NCH, 128).T)
    bpq_f = np.asarray(bpq, np.float32)
    bpqd = np.zeros((128, NCH, NH), bf)
    for m in range(NCH):
        for half in range(2):
            h = 2 * m + half
            bpqd[64 * half : 64 * half + 64, m, h] = bpq_f[
                128 * m + 64 * half : 128 * m + 64 * half + 64
            ].astype(bf)
    mask_row = (
        np.ascontiguousarray(np.asarray(attention_mask, np.float32)[0, 0, 0])
        * math.sqrt(D)
    )[X].astype(bf)
    ident = np.eye(128, dtype=bf)

    common = dict(
        wq=_chunked(np.asarray(Wq)), wk=_chunked(np.asarray(Wk)),
        wv=_chunked(np.asarray(Wv)),
        wpkT=_chunked_f8(np.ascontiguousarray(np.asarray(Wpk, np.float32).T) * WPSCALE),
        wpqT=_chunked_f8(np.ascontiguousarray(np.asarray(Wpq, np.float32).T) * WPSCALE),
        bqT=bqT, bkT=bkT, bv=np.asarray(bv, np.float32).astype(bf),
        bpqd=bpqd, maskrow=np.ascontiguousarray(mask_row), ident=ident,
    )
    in_maps = []
    pos0 = np.asarray(pos_emb)[0]  # (S, S, H) f32
    for c in range(NC):
        sl = slice(c * TB, (c + 1) * TB)
        m = dict(common)
        # [g, p, tg, mm, s] = pos[t0 + 4g + tg, X_s, 128*mm + p]
        m["pos"] = (
            pos0[sl][:, X, :]
            .transpose(0, 2, 1)
            .reshape(NG, GT, NCH, 128, S)
            .transpose(0, 3, 1, 2, 4)
            .astype(f8)
        )
        m["hsTall"] = _chunked(
            np.concatenate([hsT_X, hsT_nat[:, sl]], axis=1)
        )
        in_maps.append(m)
    return in_maps


def kernel(**inputs):
    global _NC_CACHE
    if _NC_CACHE is None:
        _NC_CACHE = build_module()
    nc = _NC_CACHE
    in_maps = _prep_inputs(**inputs)
    res = run_bass_kernel_spmd(nc, in_maps, core_ids=list(range(NC)))
    out = np.concatenate([r["out"] for r in res.results], axis=0)
    return out.reshape(1, S, H).astype(np.float32)


# revision 69
# speedup vs baseline: 1.0189x; 1.0189x over previous
"""Disentangled self-attention (DeBERTa-style) Trainium2 kernel, 8 NeuronCores.

Math restructuring (same as the 172us baseline): project q/k through
Wpk/Wpq instead of projecting pos_emb, so the single fp8 read of pos_emb
is the dominant data movement:

    c2p[h,i,j] = sum_c qpk[h,i,c] * pos[i,j,c]
    p2c[h,i,j] = sum_c kpq[h,j,c] * pos[j,i,c]   (+ k.bpq colbias term)

Each core owns 48 query rows; it computes its own c2p rows and the p2c
COLUMNS for every other core from the same pos read, exchanged via
AllToAll in 3 chunks.

Scheduling redesign vs the 172us baseline (trace-driven):
  * the collectives (dummy warm-up a2a + 3 real chunks) are the ONLY
    instructions on the gpsimd queue, and the dummy triggers at ~1us.
    Previously the blocking collective_compute stalled 48 per-slab
    SWDGE stores queued behind it, stretching the loop to ~135us.
  * all a2a-consuming work (g2 loads, p2c transposes, softmax) comes
    AFTER the full loop in per-engine program order; previously
    g2_load(0) sat at t=41 in the scalar queue and stalled the last
    loop slabs on AllToAll #0.
  * score columns are stored in a host-side permuted order
    X_n = (n%8)*48 + n//8, so a2a chunk k completes a CONTIGUOUS block
    of 8*n_slabs columns.  exp / probs-transpose / probs@v then run
    incrementally per chunk (unnormalized accumulate, one 1/sum scale
    at the end), hiding the old 33us tensor-bound tail inside the a2a
    waits; only the last 64 columns' work is exposed.
  * c2p rows ride SBUF->SBUF HWDGE extracts straight out of the cast
    staging tile (no DRAM round trip); the colbias broadcast is a K=1
    ones-matmul accumulated into the c2c PSUM (no SWDGE broadcast).
"""

import sys

sys.path.insert(0, "/opt/trn_rl_repo")

import math
import numpy as np
import ml_dtypes

import concourse.bass as bass
import concourse.bacc as bacc
import concourse.mybir as mybir
import concourse.tile as tile
from concourse.bass_utils import run_bass_kernel_spmd

BF16 = mybir.dt.bfloat16
F8E3 = mybir.dt.float8e3
F32 = mybir.dt.float32
AF = mybir.ActivationFunctionType
ADD = mybir.AluOpType.add

S = 384
H = 768
NH = 12
D = 64
NC = 8
TB = S // NC  # 48 rows per core
NCH = H // 128  # 6 chunks of the hidden dim
GT = 4  # t-slabs per pos DMA group (= PE column-tile width)
NG = TB // GT  # 12 groups
CHUNKS = [(0, 22), (22, 22), (44, 4)]  # a2a chunks: (t_off, n_slabs)
# Wpk/Wpq are ~N(0, 0.02^2): prescaled by 2^6 on the host so fp8 e3m4
# stays in its normal range; compensated in the exp scale (both stg
# halves are qkp-derived).
WPSCALE = 64.0


def build_module():
    nc = bacc.Bacc(trn_type="TRN2", num_devices=NC, debug=False)

    # ---- I/O ----
    pos_d = nc.dram_tensor("pos", [NG, 128, GT, NCH, S], F8E3, kind="ExternalInput")
    hsTall_d = nc.dram_tensor("hsTall", [128, NCH, S + TB], BF16, kind="ExternalInput")
    wq_d = nc.dram_tensor("wq", [128, NCH, H], BF16, kind="ExternalInput")
    wk_d = nc.dram_tensor("wk", [128, NCH, H], BF16, kind="ExternalInput")
    wv_d = nc.dram_tensor("wv", [128, NCH, H], BF16, kind="ExternalInput")
    wpkT_d = nc.dram_tensor("wpkT", [128, NCH, H], F8E3, kind="ExternalInput")
    wpqT_d = nc.dram_tensor("wpqT", [128, NCH, H], F8E3, kind="ExternalInput")
    bqT_d = nc.dram_tensor("bqT", [128, NCH], F32, kind="ExternalInput")
    bkT_d = nc.dram_tensor("bkT", [128, NCH], F32, kind="ExternalInput")
    bv_d = nc.dram_tensor("bv", [H], BF16, kind="ExternalInput")
    bpqd_d = nc.dram_tensor("bpqd", [128, NCH, NH], BF16, kind="ExternalInput")
    mask_d = nc.dram_tensor("maskrow", [S], BF16, kind="ExternalInput")
    ident_d = nc.dram_tensor("ident", [128, 128], BF16, kind="ExternalInput")
    out_d = nc.dram_tensor("out", [TB, H], F32, kind="ExternalOutput")

    with tile.TileContext(nc) as tc:
        with (
            tc.tile_pool(name="const", bufs=1) as cpool,
            tc.tile_pool(name="work", bufs=1) as wpool,
            tc.tile_pool(name="posT", bufs=3) as ppool,
            tc.tile_pool(name="a2asb", bufs=1) as apool,
            tc.tile_pool(name="g2p", bufs=1) as gpool,
            tc.tile_pool(name="psum", bufs=3, space="PSUM") as pspool,
            tc.tile_pool(name="psum2", bufs=5, space="PSUM") as ps2pool,
            tc.tile_pool(name="dram", bufs=1, space="DRAM") as dpool,
        ):
            # ---- startup DMAs.  pos rides the Sync ring; weights ride the
            # Scalar HWDGE ring ordered by need-time.  The gpsimd queue
            # carries ONLY the broadcasts it alone can do, then the
            # collectives -- nothing may queue behind a blocking
            # collective_compute.
            ident = cpool.tile([128, 128], BF16, tag="ident")
            wq = cpool.tile([128, NCH, H], BF16, tag="wq")
            wk = cpool.tile([128, NCH, H], BF16, tag="wk")
            wpkT = cpool.tile([128, NCH, H], F8E3, tag="wpkT")
            wpqT = cpool.tile([128, NCH, H], F8E3, tag="wpqT")
            hsTall = cpool.tile([128, NCH, S + TB], BF16, tag="hsTall")
            wv = cpool.tile([128, NCH, H], BF16, tag="wv")
            bqT = cpool.tile([128, NCH], F32, tag="bqT")
            bkT = cpool.tile([128, NCH], F32, tag="bkT")
            bpqd = cpool.tile([128, NCH, NH], BF16, tag="bpqd")
            mask_p0 = cpool.tile([1, S], BF16, tag="mask_p0")
            nc.sync.dma_start(wq[:], wq_d[:])
            nc.sync.dma_start(hsTall[:], hsTall_d[:])
            nc.sync.dma_start(wk[:], wk_d[:])

            # ---- dummy-collective staging FIRST (scalar ring head), so
            # the gpsimd dummy AllToAll triggers at ~1.5us and pays the
            # ~40-50us barrier + first-collective cost overlapped with
            # the startup DMAs.  gpsimd carries ONLY collective_computes
            # -- nothing may queue behind a blocking collective, and the
            # ~10us SWDGE descriptor-gen of a broadcast ahead of the
            # trigger costs the whole chain that delay.
            qbd = wpool.tile([128, NCH, 2 * TB], BF16, tag="qbd")
            kbd = wpool.tile([128, NCH, 2 * TB], BF16, tag="kbd")
            ones48 = cpool.tile([1, TB], BF16, tag="ones48")
            onesNH = cpool.tile([1, NH], BF16, tag="onesNH")
            ones128 = cpool.tile([1, 128], BF16, tag="ones128")
            bv_p0 = cpool.tile([1, H], BF16, tag="bv_p0")
            # No warm-up dummy collective: chunk0's data is staged by
            # ~30us, well before the ~65-95us ncfw first-collective
            # floor, so the first real AllToAll absorbs the barrier +
            # launch skew itself.
            nc.vector.memset(qbd[:], 0.0)
            nc.vector.memset(kbd[:], 0.0)
            nc.vector.memset(ones48[:], 1.0)
            nc.vector.memset(onesNH[:], 1.0)
            nc.vector.memset(ones128[:], 1.0)

            # first three pos groups issue ahead of ident (transposes need
            # ident only post-loop); later groups issue from the loop.
            NPRE = 3
            posT_pre = []
            for g in range(NPRE):
                pt = ppool.tile([128, GT, NCH, S], F8E3, tag="posT", name="posT")
                nc.sync.dma_start(pt[:], pos_d[g])
                posT_pre.append(pt)
            nc.sync.dma_start(ident[:], ident_d[:])

            # scalar ring ordered by need-time: tiny biases, then the qkp
            # weights (the whole PE backlog -- and with it the pos-buffer
            # recycling -- waits on these), then the later-need hsT/wv.
            nc.scalar.dma_start(bqT[:], bqT_d[:])
            nc.scalar.dma_start(bkT[:], bkT_d[:])
            nc.scalar.dma_start(wpkT[:], wpkT_d[:])
            nc.scalar.dma_start(wpqT[:], wpqT_d[:])
            nc.scalar.dma_start(bpqd[:], bpqd_d[:])
            nc.scalar.dma_start(bv_p0[:], bv_d[:])
            nc.scalar.dma_start(mask_p0[:], mask_d[:])
            nc.scalar.dma_start(wv[:], wv_d[:])

            # ---- PE warm-up junk so HAM unthrottles during the DMA wait
            # (K=1 on the memset ones vector -- no dependency on loads)
            psw = ps2pool.tile([128, 128], F32, tag="ps2")
            for _ in range(50):
                nc.tensor.matmul(psw[:], ones128[:], ones128[:])

            # ---- own-row projections qT_own / kT_own ----
            qTo = wpool.tile([128, NCH, TB], BF16, tag="qTo")
            for m in range(NCH):
                pso = ps2pool.tile([128, TB], F32, tag="ps2")
                for c in range(NCH):
                    nc.tensor.matmul(
                        pso[:], wq[:, c, m * 128 : (m + 1) * 128], hsTall[:, c, S : S + TB],
                        start=(c == 0), stop=(c == NCH - 1),
                    )
                nc.vector.tensor_scalar_add(qTo[:, m, :], pso[:], bqT[:, m : m + 1])

            # ---- k projection: all X-ordered columns AND the own rows in
            # one matmul per (m, c) (hsTall carries both column sets), so
            # no separate kT filler units sit in the loop's PE FIFO.
            kTall = wpool.tile([128, NCH, S + TB], BF16, tag="kTall")
            for m in range(NCH):
                psk = ps2pool.tile([128, S + TB], F32, tag="ps2")
                for c in range(NCH):
                    nc.tensor.matmul(
                        psk[:], wk[:, c, m * 128 : (m + 1) * 128], hsTall[:, c, :],
                        start=(c == 0), stop=(c == NCH - 1),
                    )
                nc.vector.tensor_scalar_add(kTall[:, m, :], psk[:], bkT[:, m : m + 1])

            # ---- block-diagonal q/k for the per-head pos projections ----
            for mh in range(NCH):
                nc.vector.tensor_copy(qbd[0:64, mh, 0:96:2], qTo[0:64, mh, :])
                nc.vector.tensor_copy(qbd[64:128, mh, 1:96:2], qTo[64:128, mh, :])
                nc.vector.tensor_copy(kbd[0:64, mh, 0:96:2], kTall[0:64, mh, S : S + TB])
                nc.vector.tensor_copy(kbd[64:128, mh, 1:96:2], kTall[64:128, mh, S : S + TB])

            # ---- qkp[128, m, t, 24]: cols 0:12 kpq (p2c side), 12:24 qpk
            # (c2p side).  p2c occupies matmul-output rows 32j+0:12 so the
            # strided deinterleave copy reads at a legal engine base; the
            # c2p rows 32j+12:24 leave via DMA (base-12 DMA reads legal).
            qkp = wpool.tile([128, NCH, TB, 2 * NH], BF16, tag="qkp")
            for m in range(NCH):
                for mh in range(NCH):
                    ps1 = ps2pool.tile([128, 2 * TB], F32, tag="ps2")
                    nc.tensor.matmul(
                        ps1[:], wpkT[:, mh, m * 128 : (m + 1) * 128], qbd[:, mh, :]
                    )
                    src1 = ps1[:].rearrange("p (t two) -> p t two", two=2)
                    if mh % 2 == 0:
                        nc.scalar.activation(
                            qkp[:, m, :, NH + 2 * mh : NH + 2 * mh + 2], src1, AF.Copy
                        )
                    else:
                        nc.vector.tensor_copy(
                            qkp[:, m, :, NH + 2 * mh : NH + 2 * mh + 2], src1
                        )
                    ps2 = ps2pool.tile([128, 2 * TB], F32, tag="ps2")
                    nc.tensor.matmul(
                        ps2[:], wpqT[:, mh, m * 128 : (m + 1) * 128], kbd[:, mh, :]
                    )
                    src2 = ps2[:].rearrange("p (t two) -> p t two", two=2)
                    if mh % 2 == 0:
                        nc.vector.tensor_copy(
                            qkp[:, m, :, 2 * mh : 2 * mh + 2], src2
                        )
                    else:
                        nc.scalar.activation(
                            qkp[:, m, :, 2 * mh : 2 * mh + 2], src2, AF.Copy
                        )

            # ---- tiles for the main loop + tail ----
            # per-head v columns with a ones column appended: the probs@v
            # matmul's 65th output column is then the softmax partial sum
            # for free (no ACT accum_out / READ_ACCUMULATOR per head).
            v_sb = wpool.tile([128, 3, NH, D + 1], BF16, tag="v_sb")
            nc.vector.memset(v_sb[:, :, :, D : D + 1], 1.0)
            scores = wpool.tile([TB, NH, S], F32, tag="scores")
            colbias = wpool.tile([NH, S], BF16, tag="colbias")
            cbp0 = wpool.tile([1, NH, S], BF16, tag="cbp0")
            # c2p rows in plain stream (X) order -> contiguous score add.
            # One DRAM store per 4-slab group (a dma_start costs ~0.8us of
            # ENGINE time regardless of size, so 12 batched stores beat 48
            # per-slab SBUF->SBUF extracts), reloaded in 2 halves.
            c2p_rows = wpool.tile([TB, NH, S], BF16, tag="c2p_rows")
            c2p_dram = dpool.tile([TB, NH, S], BF16, name="c2p_dram")
            # p2c send staging [h, dest, t_local, i_local], filled by a
            # strided deinterleave copy from the per-group cast tile,
            # staged per chunk to a2a_in DRAM.
            comb = [None, None, None]
            a2a_in = [
                dpool.tile([NC, NH, n, TB], BF16, name=f"a2a_in{k}")
                for k, (off, n) in enumerate(CHUNKS)
            ]
            a2a_out = [
                dpool.tile([NC, NH, n, TB], BF16, name=f"a2a_out{k}")
                for k, (off, n) in enumerate(CHUNKS)
            ]
            g2 = [None, None, None]

            def alloc_comb(k):
                n = CHUNKS[k][1]
                tag = "a2aAB" if k < 2 else "a2aC"
                comb[k] = apool.tile(
                    [NH, NC, n, TB], BF16, tag=tag, name=f"comb{k}"
                )

            def v_unit(jc, nh):
                def run():
                    ps = ps2pool.tile([128, S], F32, tag="ps2")
                    for c in range(NCH):
                        nc.tensor.matmul(
                            ps[:],
                            hsTall[:, c, jc * 128 : (jc + 1) * 128],
                            wv[:, c, nh * S : (nh + 1) * S],
                            start=(c == 0), stop=False,
                        )
                    # bias broadcast across the 128 j-rows via a K=1
                    # ones-matmul accumulate (no SWDGE broadcast needed).
                    nc.tensor.matmul(
                        ps[:], ones128[:], bv_p0[:, nh * S : (nh + 1) * S],
                        start=False, stop=True,
                    )
                    nc.scalar.activation(
                        v_sb[:, jc, 6 * nh : 6 * (nh + 1), 0:D],
                        ps[:].rearrange("p (h d) -> p h d", h=6),
                        AF.Copy,
                    )
                return run

            def kb_unit():
                # colbias[h, j] = bpq_h . k_j  (+ mask*sqrt(D), via a K=1
                # ones-matmul).  Copied to partition 0 so c2c can add it
                # with another K=1 accumulate -- no SWDGE broadcast.
                pskb = ps2pool.tile([NH, S], F32, tag="ps2")
                nc.tensor.matmul(
                    pskb[:], onesNH[:], mask_p0[:], start=True, stop=False,
                )
                for m in range(NCH):
                    nc.tensor.matmul(
                        pskb[:], bpqd[:, m, :], kTall[:, m, 0:S],
                        start=False, stop=(m == NCH - 1),
                    )
                nc.vector.tensor_copy(colbias[:], pskb[:])
                nc.scalar.dma_start(cbp0[:], colbias[:])

            def c2c_unit(h):
                def run():
                    mh, oh = h // 2, (h % 2) * 64
                    ps = ps2pool.tile([TB, S], F32, tag="ps2")
                    nc.tensor.matmul(
                        ps[:], qTo[oh : oh + 64, mh, :], kTall[oh : oh + 64, mh, 0:S],
                        start=True, stop=False,
                    )
                    nc.tensor.matmul(
                        ps[:], ones48[:], cbp0[:, h, :], start=False, stop=True,
                    )
                    if h % 2 == 0:
                        nc.scalar.activation(scores[:, h, :], ps[:], AF.Copy)
                    else:
                        nc.vector.tensor_copy(scores[:, h, :], ps[:])
                return run

            def c2p_reload(half):
                def run():
                    lo, hi = (0, 32) if half == 0 else (32, TB)
                    nc.scalar.dma_start(c2p_rows[lo:hi], c2p_dram[lo:hi])
                return run

            def exp_unit(h):
                # probs = exp(c2c + colbias); the c2p / p2c factors are
                # exp'd at the per-slab cast and MULTIPLIED in afterwards
                # (exp(a+b+c) = exp(a)exp(b)exp(c)), so nothing in the
                # softmax waits on the AllToAll except the final product.
                def run():
                    nc.scalar.activation(
                        probs[:, h, :], scores[:, h, :], AF.Exp, scale=isqd
                    )
                return run

            def c2p_mul(half):
                def run():
                    lo, hi = (0, 32) if half == 0 else (32, TB)
                    nc.vector.tensor_tensor(
                        probs[lo:hi], probs[lo:hi], c2p_rows[lo:hi],
                        op=mybir.AluOpType.mult,
                    )
                return run

            def stage_a2a(k):
                nc.scalar.dma_start(
                    a2a_in[k][:].rearrange("d h t i -> h d t i"),
                    comb[k][:],
                )

            def cc_issue(k):
                nc.gpsimd.collective_compute(
                    "AllToAll",
                    mybir.AluOpType.bypass,
                    replica_groups=[list(range(NC))],
                    ins=[a2a_in[k].opt()],
                    outs=[a2a_out[k].opt()],
                )

            def g2_load(k):
                n = CHUNKS[k][1]
                tag = "g2AB" if k < 2 else "g2C"
                g2[k] = gpool.tile(
                    [NC * NH, n, TB], BF16, tag=tag, name=f"g2_{k}"
                )
                nc.scalar.dma_start(
                    g2[k][:],
                    a2a_out[k][:].rearrange("d h t i -> (d h) t i"),
                )

            sums = wpool.tile([TB, NH], F32, tag="sums")
            recip = wpool.tile([TB, NH], F32, tag="recip")
            probs = wpool.tile([TB, NH, S], BF16, tag="probs")
            ptile = wpool.tile([128, 3, NH, TB], BF16, tag="ptile")
            # out_acc[:, h, 0:64] accumulates probs@v; column 64 (from the
            # appended ones column of v) accumulates the softmax sums.
            out_acc = wpool.tile([TB, NH, D + 1], F32, tag="out_acc")
            out_sb = wpool.tile([TB, H], F32, tag="out_sb")
            isqd = 1.0 / math.sqrt(D)

            def p2c_pair(k, tl):
                # transpose 2 slabs into PSUM and multiply the exp'd p2c
                # factors straight into probs from there (DVE reads PSUM;
                # no staging tile, no drain copy).
                off, n = CHUNKS[k]
                pst2 = ps2pool.tile([TB, 2, NC * NH], BF16, tag="ps2")
                for q in range(2):
                    nc.tensor.transpose(
                        pst2[:, q, :], g2[k][:, tl + q, :],
                        ident[0 : NC * NH, 0 : NC * NH],
                    )
                c0 = NC * (off + tl)
                pr = probs[:, :, c0 : c0 + 2 * NC].rearrange(
                    "i h (t s) -> i h t s", s=NC
                )
                nc.vector.tensor_tensor(
                    pr,
                    pr,
                    pst2[:].rearrange("i t (d h) -> i h t d", d=NC),
                    op=mybir.AluOpType.mult,
                )

            # ---- filler schedule keyed by global t ----
            filler = {}
            filler.setdefault(1, []).append(kb_unit)
            for h in range(NH):
                filler.setdefault(24 + h, []).append(c2c_unit(h))  # t = 24..35
            for h in range(NH):
                filler.setdefault(min(26 + h, 39), []).append(exp_unit(h))
            filler.setdefault(35, []).append(c2p_reload(0))
            filler.setdefault(41, []).append(c2p_mul(0))

            # ---- main loop over 4-slab groups ----
            for g in range(NG):
                if g < NPRE:
                    posT = posT_pre[g]
                else:
                    posT = ppool.tile([128, GT, NCH, S], F8E3, tag="posT", name="posT")
                    nc.sync.dma_start(posT[:], pos_d[g])
                ps = pspool.tile([128, S], F32, tag="ps")
                stg = ppool.tile([2 * NH, GT, S], BF16, tag="stg", name="stg")
                for j in range(GT):
                    t = GT * g + j
                    for k, (off, n) in enumerate(CHUNKS):
                        if t == off:
                            alloc_comb(k)
                for m in range(NCH):
                    for j in range(GT):
                        t = GT * g + j
                        nc.tensor.matmul(
                            ps[32 * j : 32 * j + 2 * NH, :],
                            qkp[:, m, t, :],
                            posT[:, j, m, :],
                            start=(m == 0),
                            stop=(m == NCH - 1),
                            tile_position=(0, 32 * j),
                            # the sim's zero-region tracker ignores the
                            # partition base, so the 4 disjoint column
                            # groups falsely collide; HW has_written is
                            # per-element.
                            skip_group_check=True,
                        )
                for j in range(GT):
                    t = GT * g + j
                    for k, (off, n) in enumerate(CHUNKS):
                        if off <= t < off + n:
                            break
                    tl = t - off
                    # exp'd cast off PSUM (the c2p/p2c softmax factors are
                    # exchanged and stored pre-exponentiated); the p2c half
                    # (rows 0:12) deinterleaves stream position n = i*8 + d
                    # into (dest d, dest-local row i) with a strided DVE
                    # copy, the c2p half (rows 12:24) leaves by DMA.
                    nc.scalar.activation(
                        stg[:, j, :], ps[32 * j : 32 * j + 2 * NH, :], AF.Exp,
                        scale=isqd / WPSCALE,
                    )
                    nc.vector.tensor_copy(
                        comb[k][:, :, tl, :],
                        stg[0:NH, j, :].rearrange("h (i d) -> h d i", d=NC),
                    )
                    if j == GT - 1:
                        # NOT on the sync queue: a data-dependent issue
                        # there would stall subsequent pos-prefetch issues.
                        nc.scalar.dma_start(
                            c2p_dram[GT * g : GT * (g + 1)].rearrange(
                                "t h s -> h t s"
                            ),
                            stg[NH : 2 * NH, :, :],
                        )
                    if t == CHUNKS[0][0] + CHUNKS[0][1] - 1:
                        stage_a2a(0)
                        cc_issue(0)
                    if t == CHUNKS[1][0] + CHUNKS[1][1] - 1:
                        stage_a2a(1)
                        cc_issue(1)
                    for f in filler.get(t, []):
                        f()

            # ---- after the loop: last a2a + c2p second half ----
            stage_a2a(2)
            cc_issue(2)
            c2p_reload(1)()
            c2p_mul(1)()
            for jc in range(3):
                for nh in range(2):
                    v_unit(jc, nh)()

            # ---- incremental probs@v per a2a chunk window.  probs
            # already holds exp(c2c+colbias)*exp(c2p); chunk k's exp'd
            # p2c factors multiply in, completing columns [160k, 160k+8n)
            # contiguous (X order) = 128-column block jc=k plus carry,
            # then transpose + accumulate probs@v into out_acc.
            for k, (off, n) in enumerate(CHUNKS):
                g2_load(k)
                # only the pairs covering jc-block k gate this window's
                # transposes; trailing pairs (feeding window k+1) overlap.
                need = max(0, min(n // 2, (128 * (k + 1) - NC * off + 15) // 16))
                for pr in range(need):
                    p2c_pair(k, 2 * pr)
                for h in range(NH):
                    pst = ps2pool.tile([128, TB], BF16, tag="ps2")
                    nc.tensor.transpose(
                        pst[:], probs[:, h, k * 128 : (k + 1) * 128],
                        ident[0:TB, 0:TB],
                    )
                    if h % 2 == 0:
                        nc.vector.tensor_copy(ptile[:, k, h, :], pst[:])
                    else:
                        nc.scalar.activation(ptile[:, k, h, :], pst[:], AF.Copy)
                    psc = ps2pool.tile([TB, D + 1], F32, tag="ps2")
                    nc.tensor.matmul(
                        psc[:], ptile[:, k, h, :], v_sb[:, k, h, :],
                    )
                    if k == 0:
                        if h % 2 == 0:
                            nc.scalar.activation(out_acc[:, h, :], psc[:], AF.Copy)
                        else:
                            nc.vector.tensor_copy(out_acc[:, h, :], psc[:])
                    else:
                        nc.vector.tensor_tensor(
                            out_acc[:, h, :], out_acc[:, h, :], psc[:], op=ADD
                        )
                    if k == 2:
                        # finalize per head as soon as its last PV lands
                        nc.vector.reciprocal(
                            recip[:, h : h + 1], out_acc[:, h, D : D + 1]
                        )
                        nc.scalar.activation(
                            out_sb[:, h * D : (h + 1) * D],
                            out_acc[:, h, 0:D], AF.Copy,
                            scale=recip[:, h : h + 1],
                        )
                for pr in range(need, n // 2):
                    p2c_pair(k, 2 * pr)

            nc.sync.dma_start(out_d[:], out_sb[:])

    nc.compile()
    return nc


_NC_CACHE = None


def _chunked(w):
    """[H, X] f32 -> [128, NCH, X] bf16 with [p, m, x] = w[128m+p, x]."""
    bf = ml_dtypes.bfloat16
    X = w.shape[1]
    return np.ascontiguousarray(
        np.asarray(w, np.float32).reshape(NCH, 128, X).transpose(1, 0, 2)
    ).astype(bf)


def _chunked_f8(w):
    f8 = ml_dtypes.float8_e3m4
    X = w.shape[1]
    return np.ascontiguousarray(
        np.asarray(w, np.float32).reshape(NCH, 128, X).transpose(1, 0, 2)
    ).astype(f8)


def _prep_inputs(hidden_states, attention_mask, pos_emb, Wq, bq, Wk, bk, Wv, bv,
                 Wpk, bpk, Wpq, bpq):
    bf = ml_dtypes.bfloat16
    f8 = ml_dtypes.float8_e3m4
    # column permutation: stream position n holds key index X_n so that
    # a2a chunk k completes a contiguous block of score columns.
    X = (np.arange(S) % NC) * TB + np.arange(S) // NC
    hs = np.ascontiguousarray(np.asarray(hidden_states, np.float32)[0])  # (S, H)
    hsT_nat = np.ascontiguousarray(hs.T)  # (H, S) f32, natural order
    hsT_X = np.ascontiguousarray(hsT_nat[:, X])
    bqT = np.ascontiguousarray(np.asarray(bq, np.float32).reshape(NCH, 128).T)
    bkT = np.ascontiguousarray(np.asarray(bk, np.float32).reshape(NCH, 128).T)
    bpq_f = np.asarray(bpq, np.float32)
    bpqd = np.zeros((128, NCH, NH), bf)
    for m in range(NCH):
        for half in range(2):
            h = 2 * m + half
            bpqd[64 * half : 64 * half + 64, m, h] = bpq_f[
                128 * m + 64 * half : 128 * m + 64 * half + 64
            ].astype(bf)
    mask_row = (
        np.ascontiguousarray(np.asarray(attention_mask, np.float32)[0, 0, 0])
        * math.sqrt(D)
    )[X].astype(bf)
    ident = np.eye(128, dtype=bf)

    common = dict(
        wq=_chunked(np.asarray(Wq)), wk=_chunked(np.asarray(Wk)),
        wv=_chunked(np.asarray(Wv)),
        wpkT=_chunked_f8(np.ascontiguousarray(np.asarray(Wpk, np.float32).T) * WPSCALE),
        wpqT=_chunked_f8(np.ascontiguousarray(np.asarray(Wpq, np.float32).T) * WPSCALE),
        bqT=bqT, bkT=bkT, bv=np.asarray(bv, np.float32).astype(bf),
        bpqd=bpqd, maskrow=np.ascontiguousarray(mask_row), ident=ident,
    )
    in_maps = []
    pos0 = np.asarray(pos_emb)[0]  # (S, S, H) f32
    for c in range(NC):
        sl = slice(c * TB, (c + 1) * TB)
        m = dict(common)
        # [g, p, tg, mm, s] = pos[t0 + 4g + tg, X_s, 128*mm + p]
        m["pos"] = (
            pos0[sl][:, X, :]
            .transpose(0, 2, 1)
            .reshape(NG, GT, NCH, 128, S)
            .transpose(0, 3, 1, 2, 4)
            .astype(f8)
        )
        m["hsTall"] = _chunked(
            np.concatenate([hsT_X, hsT_nat[:, sl]], axis=1)
        )
        in_maps.append(m)
    return in_maps


def kernel(**inputs):
    global _NC_CACHE
    if _NC_CACHE is None:
        _NC_CACHE = build_module()
    nc = _NC_CACHE
    in_maps = _prep_inputs(**inputs)
    res = run_bass_kernel_spmd(nc, in_maps, core_ids=list(range(NC)))
    out = np.concatenate([r["out"] for r in res.results], axis=0)
    return out.reshape(1, S, H).astype(np.float32)


# revision 70
# speedup vs baseline: 1.0404x; 1.0211x over previous
"""Disentangled self-attention (DeBERTa-style) Trainium2 kernel, 8 NeuronCores.

Math restructuring (same as the 172us baseline): project q/k through
Wpk/Wpq instead of projecting pos_emb, so the single fp8 read of pos_emb
is the dominant data movement:

    c2p[h,i,j] = sum_c qpk[h,i,c] * pos[i,j,c]
    p2c[h,i,j] = sum_c kpq[h,j,c] * pos[j,i,c]   (+ k.bpq colbias term)

Each core owns 48 query rows; it computes its own c2p rows and the p2c
COLUMNS for every other core from the same pos read, exchanged via
AllToAll in 3 chunks.

Scheduling redesign vs the 172us baseline (trace-driven):
  * the collectives (dummy warm-up a2a + 3 real chunks) are the ONLY
    instructions on the gpsimd queue, and the dummy triggers at ~1us.
    Previously the blocking collective_compute stalled 48 per-slab
    SWDGE stores queued behind it, stretching the loop to ~135us.
  * all a2a-consuming work (g2 loads, p2c transposes, softmax) comes
    AFTER the full loop in per-engine program order; previously
    g2_load(0) sat at t=41 in the scalar queue and stalled the last
    loop slabs on AllToAll #0.
  * score columns are stored in a host-side permuted order
    X_n = (n%8)*48 + n//8, so a2a chunk k completes a CONTIGUOUS block
    of 8*n_slabs columns.  exp / probs-transpose / probs@v then run
    incrementally per chunk (unnormalized accumulate, one 1/sum scale
    at the end), hiding the old 33us tensor-bound tail inside the a2a
    waits; only the last 64 columns' work is exposed.
  * c2p rows ride SBUF->SBUF HWDGE extracts straight out of the cast
    staging tile (no DRAM round trip); the colbias broadcast is a K=1
    ones-matmul accumulated into the c2c PSUM (no SWDGE broadcast).
"""

import sys

sys.path.insert(0, "/opt/trn_rl_repo")

import math
import numpy as np
import ml_dtypes

import concourse.bass as bass
import concourse.bacc as bacc
import concourse.mybir as mybir
import concourse.tile as tile
from concourse.bass_utils import run_bass_kernel_spmd

BF16 = mybir.dt.bfloat16
F8E3 = mybir.dt.float8e3
F32 = mybir.dt.float32
AF = mybir.ActivationFunctionType
ADD = mybir.AluOpType.add

S = 384
H = 768
NH = 12
D = 64
NC = 8
TB = S // NC  # 48 rows per core
NCH = H // 128  # 6 chunks of the hidden dim
GT = 4  # t-slabs per pos DMA group (= PE column-tile width)
NG = TB // GT  # 12 groups
CHUNKS = [(0, 22), (22, 22), (44, 4)]  # a2a chunks: (t_off, n_slabs)
# Wpk/Wpq are ~N(0, 0.02^2): prescaled by 2^6 on the host so fp8 e3m4
# stays in its normal range; compensated in the exp scale (both stg
# halves are qkp-derived).
WPSCALE = 64.0


def build_module():
    nc = bacc.Bacc(trn_type="TRN2", num_devices=NC, debug=False)

    # ---- I/O ----
    pos_d = nc.dram_tensor("pos", [NG, 128, GT, NCH, S], F8E3, kind="ExternalInput")
    hsT_d = nc.dram_tensor("hsT", [128, NCH, S], BF16, kind="ExternalInput")
    hsTo_d = nc.dram_tensor("hsTo", [128, NCH, TB], BF16, kind="ExternalInput")
    wq_d = nc.dram_tensor("wq", [128, NCH, H], BF16, kind="ExternalInput")
    wk_d = nc.dram_tensor("wk", [128, NCH, H], BF16, kind="ExternalInput")
    wv_d = nc.dram_tensor("wv", [128, NCH, H], BF16, kind="ExternalInput")
    wpkT_d = nc.dram_tensor("wpkT", [128, NCH, H], F8E3, kind="ExternalInput")
    wpqT_d = nc.dram_tensor("wpqT", [128, NCH, H], F8E3, kind="ExternalInput")
    bqT_d = nc.dram_tensor("bqT", [128, NCH], F32, kind="ExternalInput")
    bkT_d = nc.dram_tensor("bkT", [128, NCH], F32, kind="ExternalInput")
    bv_d = nc.dram_tensor("bv", [H], BF16, kind="ExternalInput")
    bpqd_d = nc.dram_tensor("bpqd", [128, NCH, NH], BF16, kind="ExternalInput")
    mask_d = nc.dram_tensor("maskrow", [S], BF16, kind="ExternalInput")
    ident_d = nc.dram_tensor("ident", [128, 128], BF16, kind="ExternalInput")
    out_d = nc.dram_tensor("out", [TB, H], F32, kind="ExternalOutput")

    with tile.TileContext(nc) as tc:
        with (
            tc.tile_pool(name="const", bufs=1) as cpool,
            tc.tile_pool(name="work", bufs=1) as wpool,
            tc.tile_pool(name="posT", bufs=3) as ppool,
            tc.tile_pool(name="a2asb", bufs=1) as apool,
            tc.tile_pool(name="g2p", bufs=1) as gpool,
            tc.tile_pool(name="psum", bufs=3, space="PSUM") as pspool,
            tc.tile_pool(name="psum2", bufs=5, space="PSUM") as ps2pool,
            tc.tile_pool(name="dram", bufs=1, space="DRAM") as dpool,
        ):
            # ---- startup DMAs.  pos rides the Sync ring; weights ride the
            # Scalar HWDGE ring ordered by need-time.  The gpsimd queue
            # carries ONLY the broadcasts it alone can do, then the
            # collectives -- nothing may queue behind a blocking
            # collective_compute.
            ident = cpool.tile([128, 128], BF16, tag="ident")
            wq = cpool.tile([128, NCH, H], BF16, tag="wq")
            wk = cpool.tile([128, NCH, H], BF16, tag="wk")
            wpkT = cpool.tile([128, NCH, H], F8E3, tag="wpkT")
            wpqT = cpool.tile([128, NCH, H], F8E3, tag="wpqT")
            hsTo = cpool.tile([128, NCH, TB], BF16, tag="hsTo")
            hsT = cpool.tile([128, NCH, S], BF16, tag="hsT")
            wv = cpool.tile([128, NCH, H], BF16, tag="wv")
            bqT = cpool.tile([128, NCH], F32, tag="bqT")
            bkT = cpool.tile([128, NCH], F32, tag="bkT")
            bpqd = cpool.tile([128, NCH, NH], BF16, tag="bpqd")
            mask_p0 = cpool.tile([1, S], BF16, tag="mask_p0")
            nc.sync.dma_start(wq[:], wq_d[:])
            nc.sync.dma_start(hsTo[:], hsTo_d[:])
            nc.sync.dma_start(wk[:], wk_d[:])

            # ---- dummy-collective staging FIRST (scalar ring head), so
            # the gpsimd dummy AllToAll triggers at ~1.5us and pays the
            # ~40-50us barrier + first-collective cost overlapped with
            # the startup DMAs.  gpsimd carries ONLY collective_computes
            # -- nothing may queue behind a blocking collective, and the
            # ~10us SWDGE descriptor-gen of a broadcast ahead of the
            # trigger costs the whole chain that delay.
            qbd = wpool.tile([128, NCH, 2 * TB], BF16, tag="qbd")
            kbd = wpool.tile([128, NCH, 2 * TB], BF16, tag="kbd")
            ones48 = cpool.tile([1, TB], BF16, tag="ones48")
            onesNH = cpool.tile([1, NH], BF16, tag="onesNH")
            ones128 = cpool.tile([1, 128], BF16, tag="ones128")
            bv_p0 = cpool.tile([1, H], BF16, tag="bv_p0")
            # No warm-up dummy collective: chunk0's data is staged by
            # ~30us, well before the ~65-95us ncfw first-collective
            # floor, so the first real AllToAll absorbs the barrier +
            # launch skew itself.
            nc.vector.memset(qbd[:], 0.0)
            nc.vector.memset(kbd[:], 0.0)
            nc.vector.memset(ones48[:], 1.0)
            nc.vector.memset(onesNH[:], 1.0)
            nc.vector.memset(ones128[:], 1.0)

            # first three pos groups issue ahead of ident (transposes need
            # ident only post-loop); later groups issue from the loop.
            NPRE = 3
            posT_pre = []
            for g in range(NPRE):
                pt = ppool.tile([128, GT, NCH, S], F8E3, tag="posT", name="posT")
                nc.sync.dma_start(pt[:], pos_d[g])
                posT_pre.append(pt)
            nc.sync.dma_start(ident[:], ident_d[:])

            # scalar ring ordered by need-time: tiny biases, then the qkp
            # weights (the whole PE backlog -- and with it the pos-buffer
            # recycling -- waits on these), then the later-need hsT/wv.
            nc.scalar.dma_start(bqT[:], bqT_d[:])
            nc.scalar.dma_start(bkT[:], bkT_d[:])
            nc.scalar.dma_start(wpkT[:], wpkT_d[:])
            nc.scalar.dma_start(wpqT[:], wpqT_d[:])
            nc.scalar.dma_start(hsT[:], hsT_d[:])
            nc.scalar.dma_start(bpqd[:], bpqd_d[:])
            nc.scalar.dma_start(bv_p0[:], bv_d[:])
            nc.scalar.dma_start(mask_p0[:], mask_d[:])
            nc.scalar.dma_start(wv[:], wv_d[:])

            # ---- PE warm-up junk so HAM unthrottles during the DMA wait
            # (K=1 on the memset ones vector -- no dependency on loads)
            psw = ps2pool.tile([128, 128], F32, tag="ps2")
            for _ in range(50):
                nc.tensor.matmul(psw[:], ones128[:], ones128[:])

            # ---- own-row projections qT_own / kT_own ----
            qTo = wpool.tile([128, NCH, TB], BF16, tag="qTo")
            kTo = wpool.tile([128, NCH, TB], BF16, tag="kTo")
            for m in range(NCH):
                pso = ps2pool.tile([128, TB], F32, tag="ps2")
                for c in range(NCH):
                    nc.tensor.matmul(
                        pso[:], wq[:, c, m * 128 : (m + 1) * 128], hsTo[:, c, :],
                        start=(c == 0), stop=(c == NCH - 1),
                    )
                nc.vector.tensor_scalar_add(qTo[:, m, :], pso[:], bqT[:, m : m + 1])
                psk = ps2pool.tile([128, TB], F32, tag="ps2")
                for c in range(NCH):
                    nc.tensor.matmul(
                        psk[:], wk[:, c, m * 128 : (m + 1) * 128], hsTo[:, c, :],
                        start=(c == 0), stop=(c == NCH - 1),
                    )
                nc.vector.tensor_scalar_add(kTo[:, m, :], psk[:], bkT[:, m : m + 1])

            # ---- block-diagonal q/k for the per-head pos projections ----
            for mh in range(NCH):
                nc.vector.tensor_copy(qbd[0:64, mh, 0:96:2], qTo[0:64, mh, :])
                nc.vector.tensor_copy(qbd[64:128, mh, 1:96:2], qTo[64:128, mh, :])
                nc.vector.tensor_copy(kbd[0:64, mh, 0:96:2], kTo[0:64, mh, :])
                nc.vector.tensor_copy(kbd[64:128, mh, 1:96:2], kTo[64:128, mh, :])

            # ---- qkp[128, m, t, 24]: cols 0:12 kpq (p2c side), 12:24 qpk
            # (c2p side).  p2c occupies matmul-output rows 32j+0:12 so the
            # strided deinterleave copy reads at a legal engine base; the
            # c2p rows 32j+12:24 leave via DMA (base-12 DMA reads legal).
            qkp = wpool.tile([128, NCH, TB, 2 * NH], BF16, tag="qkp")
            for m in range(NCH):
                for mh in range(NCH):
                    ps1 = ps2pool.tile([128, 2 * TB], F32, tag="ps2")
                    nc.tensor.matmul(
                        ps1[:], wpkT[:, mh, m * 128 : (m + 1) * 128], qbd[:, mh, :]
                    )
                    src1 = ps1[:].rearrange("p (t two) -> p t two", two=2)
                    if mh % 2 == 0:
                        nc.scalar.activation(
                            qkp[:, m, :, NH + 2 * mh : NH + 2 * mh + 2], src1, AF.Copy
                        )
                    else:
                        nc.vector.tensor_copy(
                            qkp[:, m, :, NH + 2 * mh : NH + 2 * mh + 2], src1
                        )
                    ps2 = ps2pool.tile([128, 2 * TB], F32, tag="ps2")
                    nc.tensor.matmul(
                        ps2[:], wpqT[:, mh, m * 128 : (m + 1) * 128], kbd[:, mh, :]
                    )
                    src2 = ps2[:].rearrange("p (t two) -> p t two", two=2)
                    if mh % 2 == 0:
                        nc.vector.tensor_copy(
                            qkp[:, m, :, 2 * mh : 2 * mh + 2], src2
                        )
                    else:
                        nc.scalar.activation(
                            qkp[:, m, :, 2 * mh : 2 * mh + 2], src2, AF.Copy
                        )

            # ---- tiles for the main loop + tail ----
            kT = wpool.tile([128, NCH, S], BF16, tag="kT")
            # per-head v columns with a ones column appended: the probs@v
            # matmul's 65th output column is then the softmax partial sum
            # for free (no ACT accum_out / READ_ACCUMULATOR per head).
            v_sb = wpool.tile([128, 3, NH, D + 1], BF16, tag="v_sb")
            nc.vector.memset(v_sb[:, :, :, D : D + 1], 1.0)
            scores = wpool.tile([TB, NH, S], F32, tag="scores")
            colbias = wpool.tile([NH, S], BF16, tag="colbias")
            cbp0 = wpool.tile([1, NH, S], BF16, tag="cbp0")
            # c2p rows in plain stream (X) order -> contiguous score add.
            # One DRAM store per 4-slab group (a dma_start costs ~0.8us of
            # ENGINE time regardless of size, so 12 batched stores beat 48
            # per-slab SBUF->SBUF extracts), reloaded in 2 halves.
            c2p_rows = wpool.tile([TB, NH, S], BF16, tag="c2p_rows")
            c2p_dram = dpool.tile([TB, NH, S], BF16, name="c2p_dram")
            # p2c send staging [h, dest, t_local, i_local], filled by a
            # strided deinterleave copy from the per-group cast tile,
            # staged per chunk to a2a_in DRAM.
            comb = [None, None, None]
            a2a_in = [
                dpool.tile([NC, NH, n, TB], BF16, name=f"a2a_in{k}")
                for k, (off, n) in enumerate(CHUNKS)
            ]
            a2a_out = [
                dpool.tile([NC, NH, n, TB], BF16, name=f"a2a_out{k}")
                for k, (off, n) in enumerate(CHUNKS)
            ]
            g2 = [None, None, None]

            def alloc_comb(k):
                n = CHUNKS[k][1]
                tag = "a2aAB" if k < 2 else "a2aC"
                comb[k] = apool.tile(
                    [NH, NC, n, TB], BF16, tag=tag, name=f"comb{k}"
                )

            def kT_unit(m):
                def run():
                    ps = ps2pool.tile([128, S], F32, tag="ps2")
                    for c in range(NCH):
                        nc.tensor.matmul(
                            ps[:], wk[:, c, m * 128 : (m + 1) * 128], hsT[:, c, :],
                            start=(c == 0), stop=(c == NCH - 1),
                        )
                    nc.vector.tensor_scalar_add(kT[:, m, :], ps[:], bkT[:, m : m + 1])
                return run

            def v_unit(jc, nh):
                def run():
                    ps = ps2pool.tile([128, S], F32, tag="ps2")
                    for c in range(NCH):
                        nc.tensor.matmul(
                            ps[:],
                            hsT[:, c, jc * 128 : (jc + 1) * 128],
                            wv[:, c, nh * S : (nh + 1) * S],
                            start=(c == 0), stop=False,
                        )
                    # bias broadcast across the 128 j-rows via a K=1
                    # ones-matmul accumulate (no SWDGE broadcast needed).
                    nc.tensor.matmul(
                        ps[:], ones128[:], bv_p0[:, nh * S : (nh + 1) * S],
                        start=False, stop=True,
                    )
                    nc.scalar.activation(
                        v_sb[:, jc, 6 * nh : 6 * (nh + 1), 0:D],
                        ps[:].rearrange("p (h d) -> p h d", h=6),
                        AF.Copy,
                    )
                return run

            def kb_unit():
                # colbias[h, j] = bpq_h . k_j  (+ mask*sqrt(D), via a K=1
                # ones-matmul).  Copied to partition 0 so c2c can add it
                # with another K=1 accumulate -- no SWDGE broadcast.
                pskb = ps2pool.tile([NH, S], F32, tag="ps2")
                nc.tensor.matmul(
                    pskb[:], onesNH[:], mask_p0[:], start=True, stop=False,
                )
                for m in range(NCH):
                    nc.tensor.matmul(
                        pskb[:], bpqd[:, m, :], kT[:, m, :],
                        start=False, stop=(m == NCH - 1),
                    )
                nc.vector.tensor_copy(colbias[:], pskb[:])
                nc.scalar.dma_start(cbp0[:], colbias[:])

            def c2c_unit(h):
                def run():
                    mh, oh = h // 2, (h % 2) * 64
                    ps = ps2pool.tile([TB, S], F32, tag="ps2")
                    nc.tensor.matmul(
                        ps[:], qTo[oh : oh + 64, mh, :], kT[oh : oh + 64, mh, :],
                        start=True, stop=False,
                    )
                    nc.tensor.matmul(
                        ps[:], ones48[:], cbp0[:, h, :], start=False, stop=True,
                    )
                    if h % 2 == 0:
                        nc.scalar.activation(scores[:, h, :], ps[:], AF.Copy)
                    else:
                        nc.vector.tensor_copy(scores[:, h, :], ps[:])
                return run

            def c2p_reload(half):
                def run():
                    lo, hi = (0, 32) if half == 0 else (32, TB)
                    nc.scalar.dma_start(c2p_rows[lo:hi], c2p_dram[lo:hi])
                return run

            def exp_unit(h):
                # probs = exp(c2c + colbias); the c2p / p2c factors are
                # exp'd at the per-slab cast and MULTIPLIED in afterwards
                # (exp(a+b+c) = exp(a)exp(b)exp(c)), so nothing in the
                # softmax waits on the AllToAll except the final product.
                def run():
                    nc.scalar.activation(
                        probs[:, h, :], scores[:, h, :], AF.Exp, scale=isqd
                    )
                return run

            def c2p_mul(half):
                def run():
                    lo, hi = (0, 32) if half == 0 else (32, TB)
                    nc.vector.tensor_tensor(
                        probs[lo:hi], probs[lo:hi], c2p_rows[lo:hi],
                        op=mybir.AluOpType.mult,
                    )
                return run

            def stage_a2a(k):
                nc.scalar.dma_start(
                    a2a_in[k][:].rearrange("d h t i -> h d t i"),
                    comb[k][:],
                )

            def cc_issue(k):
                nc.gpsimd.collective_compute(
                    "AllToAll",
                    mybir.AluOpType.bypass,
                    replica_groups=[list(range(NC))],
                    ins=[a2a_in[k].opt()],
                    outs=[a2a_out[k].opt()],
                )

            def g2_load(k):
                n = CHUNKS[k][1]
                tag = "g2AB" if k < 2 else "g2C"
                g2[k] = gpool.tile(
                    [NC * NH, n, TB], BF16, tag=tag, name=f"g2_{k}"
                )
                nc.scalar.dma_start(
                    g2[k][:],
                    a2a_out[k][:].rearrange("d h t i -> (d h) t i"),
                )

            sums = wpool.tile([TB, NH], F32, tag="sums")
            recip = wpool.tile([TB, NH], F32, tag="recip")
            probs = wpool.tile([TB, NH, S], BF16, tag="probs")
            ptile = wpool.tile([128, 3, NH, TB], BF16, tag="ptile")
            # out_acc[:, h, 0:64] accumulates probs@v; column 64 (from the
            # appended ones column of v) accumulates the softmax sums.
            out_acc = wpool.tile([TB, NH, D + 1], F32, tag="out_acc")
            out_sb = wpool.tile([TB, H], F32, tag="out_sb")
            isqd = 1.0 / math.sqrt(D)

            def p2c_pair(k, tl):
                # transpose 2 slabs into PSUM and multiply the exp'd p2c
                # factors straight into probs from there (DVE reads PSUM;
                # no staging tile, no drain copy).
                off, n = CHUNKS[k]
                pst2 = ps2pool.tile([TB, 2, NC * NH], BF16, tag="ps2")
                for q in range(2):
                    nc.tensor.transpose(
                        pst2[:, q, :], g2[k][:, tl + q, :],
                        ident[0 : NC * NH, 0 : NC * NH],
                    )
                c0 = NC * (off + tl)
                pr = probs[:, :, c0 : c0 + 2 * NC].rearrange(
                    "i h (t s) -> i h t s", s=NC
                )
                nc.vector.tensor_tensor(
                    pr,
                    pr,
                    pst2[:].rearrange("i t (d h) -> i h t d", d=NC),
                    op=mybir.AluOpType.mult,
                )

            # ---- filler schedule keyed by global t ----
            filler = {}
            for m in range(NCH):
                filler.setdefault(2 * m + 1, []).append(kT_unit(m))  # t = 1..11
            filler.setdefault(13, []).append(kb_unit)
            slot = 15
            for jc in range(3):
                for nh in range(2):
                    filler.setdefault(slot, []).append(v_unit(jc, nh)); slot += 2
            for h in range(NH):
                filler.setdefault(26 + h, []).append(c2c_unit(h))  # t = 26..37
            for h in range(NH):
                filler.setdefault(min(28 + h, 39), []).append(exp_unit(h))
            filler.setdefault(35, []).append(c2p_reload(0))
            filler.setdefault(41, []).append(c2p_mul(0))

            # ---- main loop over 4-slab groups ----
            for g in range(NG):
                if g < NPRE:
                    posT = posT_pre[g]
                else:
                    posT = ppool.tile([128, GT, NCH, S], F8E3, tag="posT", name="posT")
                    nc.sync.dma_start(posT[:], pos_d[g])
                ps = pspool.tile([128, S], F32, tag="ps")
                stg = ppool.tile([2 * NH, GT, S], BF16, tag="stg", name="stg")
                for j in range(GT):
                    t = GT * g + j
                    for k, (off, n) in enumerate(CHUNKS):
                        if t == off:
                            alloc_comb(k)
                for m in range(NCH):
                    for j in range(GT):
                        t = GT * g + j
                        nc.tensor.matmul(
                            ps[32 * j : 32 * j + 2 * NH, :],
                            qkp[:, m, t, :],
                            posT[:, j, m, :],
                            start=(m == 0),
                            stop=(m == NCH - 1),
                            tile_position=(0, 32 * j),
                            # the sim's zero-region tracker ignores the
                            # partition base, so the 4 disjoint column
                            # groups falsely collide; HW has_written is
                            # per-element.
                            skip_group_check=True,
                        )
                for j in range(GT):
                    t = GT * g + j
                    for k, (off, n) in enumerate(CHUNKS):
                        if off <= t < off + n:
                            break
                    tl = t - off
                    # exp'd cast off PSUM (the c2p/p2c softmax factors are
                    # exchanged and stored pre-exponentiated); the p2c half
                    # (rows 0:12) deinterleaves stream position n = i*8 + d
                    # into (dest d, dest-local row i) with a strided DVE
                    # copy, the c2p half (rows 12:24) leaves by DMA.
                    nc.scalar.activation(
                        stg[:, j, :], ps[32 * j : 32 * j + 2 * NH, :], AF.Exp,
                        scale=isqd / WPSCALE,
                    )
                    nc.vector.tensor_copy(
                        comb[k][:, :, tl, :],
                        stg[0:NH, j, :].rearrange("h (i d) -> h d i", d=NC),
                    )
                    if j == GT - 1:
                        # NOT on the sync queue: a data-dependent issue
                        # there would stall subsequent pos-prefetch issues.
                        nc.scalar.dma_start(
                            c2p_dram[GT * g : GT * (g + 1)].rearrange(
                                "t h s -> h t s"
                            ),
                            stg[NH : 2 * NH, :, :],
                        )
                    if t == CHUNKS[0][0] + CHUNKS[0][1] - 1:
                        stage_a2a(0)
                        cc_issue(0)
                    if t == CHUNKS[1][0] + CHUNKS[1][1] - 1:
                        stage_a2a(1)
                        cc_issue(1)
                    for f in filler.get(t, []):
                        f()

            # ---- after the loop: last a2a + c2p second half ----
            stage_a2a(2)
            cc_issue(2)
            c2p_reload(1)()
            c2p_mul(1)()

            # ---- incremental probs@v per a2a chunk window.  probs
            # already holds exp(c2c+colbias)*exp(c2p); chunk k's exp'd
            # p2c factors multiply in, completing columns [160k, 160k+8n)
            # contiguous (X order) = 128-column block jc=k plus carry,
            # then transpose + accumulate probs@v into out_acc.
            for k, (off, n) in enumerate(CHUNKS):
                g2_load(k)
                # only the pairs covering jc-block k gate this window's
                # transposes; trailing pairs (feeding window k+1) overlap.
                need = max(0, min(n // 2, (128 * (k + 1) - NC * off + 15) // 16))
                for pr in range(need):
                    p2c_pair(k, 2 * pr)
                for h in range(NH):
                    pst = ps2pool.tile([128, TB], BF16, tag="ps2")
                    nc.tensor.transpose(
                        pst[:], probs[:, h, k * 128 : (k + 1) * 128],
                        ident[0:TB, 0:TB],
                    )
                    if h % 2 == 0:
                        nc.vector.tensor_copy(ptile[:, k, h, :], pst[:])
                    else:
                        nc.scalar.activation(ptile[:, k, h, :], pst[:], AF.Copy)
                    psc = ps2pool.tile([TB, D + 1], F32, tag="ps2")
                    nc.tensor.matmul(
                        psc[:], ptile[:, k, h, :], v_sb[:, k, h, :],
                    )
                    if k == 0:
                        if h % 2 == 0:
                            nc.scalar.activation(out_acc[:, h, :], psc[:], AF.Copy)
                        else:
                            nc.vector.tensor_copy(out_acc[:, h, :], psc[:])
                    else:
                        nc.vector.tensor_tensor(
                            out_acc[:, h, :], out_acc[:, h, :], psc[:], op=ADD
                        )
                    if k == 2:
                        # finalize per head as soon as its last PV lands
                        nc.vector.reciprocal(
                            recip[:, h : h + 1], out_acc[:, h, D : D + 1]
                        )
                        nc.scalar.activation(
                            out_sb[:, h * D : (h + 1) * D],
                            out_acc[:, h, 0:D], AF.Copy,
                            scale=recip[:, h : h + 1],
                        )
                for pr in range(need, n // 2):
                    p2c_pair(k, 2 * pr)

            nc.sync.dma_start(out_d[:], out_sb[:])

    nc.compile()
    return nc


_NC_CACHE = None


def _chunked(w):
    """[H, X] f32 -> [128, NCH, X] bf16 with [p, m, x] = w[128m+p, x]."""
    bf = ml_dtypes.bfloat16
    X = w.shape[1]
    return np.ascontiguousarray(
        np.asarray(w, np.float32).reshape(NCH, 128, X).transpose(1, 0, 2)
    ).astype(bf)


def _chunked_f8(w):
    f8 = ml_dtypes.float8_e3m4
    X = w.shape[1]
    return np.ascontiguousarray(
        np.asarray(w, np.float32).reshape(NCH, 128, X).transpose(1, 0, 2)
    ).astype(f8)


def _prep_inputs(hidden_states, attention_mask, pos_emb, Wq, bq, Wk, bk, Wv, bv,
                 Wpk, bpk, Wpq, bpq):
    bf = ml_dtypes.bfloat16
    f8 = ml_dtypes.float8_e3m4
    # column permutation: stream position n holds key index X_n so that
    # a2a chunk k completes a contiguous block of score columns.
    X = (np.arange(S) % NC) * TB + np.arange(S) // NC
    hs = np.ascontiguousarray(np.asarray(hidden_states, np.float32)[0])  # (S, H)
    hsT_nat = np.ascontiguousarray(hs.T)  # (H, S) f32, natural order
    hsT_X = np.ascontiguousarray(hsT_nat[:, X])
    bqT = np.ascontiguousarray(np.asarray(bq, np.float32).reshape(NCH, 128).T)
    bkT = np.ascontiguousarray(np.asarray(bk, np.float32).reshape(NCH, 128).T)
    bpq_f = np.asarray(bpq, np.float32)
    bpqd = np.zeros((128, NCH, NH), bf)
    for m in range(NCH):
        for half in range(2):
            h = 2 * m + half
            bpqd[64 * half : 64 * half + 64, m, h] = bpq_f[
                128 * m + 64 * half : 128 * m + 64 * half + 64
            ].astype(bf)
    mask_row = (
        np.ascontiguousarray(np.asarray(attention_mask, np.float32)[0, 0, 0])
        * math.sqrt(D)
    )[X].astype(bf)
    ident = np.eye(128, dtype=bf)

    common = dict(
        wq=_chunked(np.asarray(Wq)), wk=_chunked(np.asarray(Wk)),
        wv=_chunked(np.asarray(Wv)),
        hsT=_chunked(hsT_X),
        wpkT=_chunked_f8(np.ascontiguousarray(np.asarray(Wpk, np.float32).T) * WPSCALE),
        wpqT=_chunked_f8(np.ascontiguousarray(np.asarray(Wpq, np.float32).T) * WPSCALE),
        bqT=bqT, bkT=bkT, bv=np.asarray(bv, np.float32).astype(bf),
        bpqd=bpqd, maskrow=np.ascontiguousarray(mask_row), ident=ident,
    )
    in_maps = []
    pos0 = np.asarray(pos_emb)[0]  # (S, S, H) f32
    for c in range(NC):
        sl = slice(c * TB, (c + 1) * TB)
        m = dict(common)
        # [g, p, tg, mm, s] = pos[t0 + 4g + tg, X_s, 128*mm + p]
        m["pos"] = (
            pos0[sl][:, X, :]
            .transpose(0, 2, 1)
            .reshape(NG, GT, NCH, 128, S)
            .transpose(0, 3, 1, 2, 4)
            .astype(f8)
        )
        m["hsTo"] = _chunked(hsT_nat[:, sl])
        in_maps.append(m)
    return in_maps


def kernel(**inputs):
    global _NC_CACHE
    if _NC_CACHE is None:
        _NC_CACHE = build_module()
    nc = _NC_CACHE
    in_maps = _prep_inputs(**inputs)
    res = run_bass_kernel_spmd(nc, in_maps, core_ids=list(range(NC)))
    out = np.concatenate([r["out"] for r in res.results], axis=0)
    return out.reshape(1, S, H).astype(np.float32)
